# revision 26
# baseline (speedup 1.0000x reference)
"""Trainium2 Bass kernel for span-attention pooling.

Problem shapes (hardcoded):
  x: [B=2, T=512, E=1024] f32, W: [1024, 1] f32, b: [1] f32,
  start/end: [S=2048] i32.  Output: [B, S, E] f32.

Math: out[b,s,:] = sum_t m01[t,s] * q[t] * x[b,t,:] / sum_t m01[t,s] * q[t]
with q = max(exp(h + b), 1) = exp(relu(h + b)), h = x @ W, and
m01[t,s] = (start[s] <= t <= end[s]) a host-precomputed 0/1 mask.
(Equivalent to the reference's per-span softmax over head scores.)

Per core: head scores h via DVE multiply-accumulate halves, q =
max(exp(h+b),1) (ACT+DVE), xq = q * x, then on the PE
  po = m01.T @ xq   (two 512-col halves per 128-span chunk)
  Z  = m01.T @ q16  (N=1 columns)
and out = po * (1/Z) (DVE/ACT whole-group ops), stored fp16 (host
upcasts).  Only (chunk, span-chunk) matmul pairs whose mask slice is
nonzero are emitted (the compiled program is cached per live-pattern);
each span-chunk's poB half and reciprocal fire as soon as its last live
chunk accumulates.

Input packing: chunk 0's x and the replicated W interleave in one tile
([x0a | Wa | x0b | Wb | b]) loaded as two half DMAs on the sync ring, so
the first head-score op starts as soon as 256KB lands; masks follow on
the sync ring while x chunk 1 has the scalar ring to itself.

Sharding: 8 cores = (batch b in {0,1}) x (512-span group). Spans are
sorted by start on the host and split into quarters; since a span covers
at most 30 consecutive tokens, each quarter's spans live inside a window
of <= 256 tokens, so each core only loads its 256-token x slice. If an
exotic span distribution breaks the window property, the kernel falls
back to unsorted quarters with the full K=512.

PSUM bank plan (8): a0..a3 (poA), z0..z3 (Z); each poB reuses its z
bank after the reciprocal reads it. PE warm-up (the HAM clock gate needs
~3.4us of sustained activity and an idle gap resets it) runs dummy
matmuls sized to abut the first real matmul.
"""

import numpy as np

import concourse.bass as bass
import concourse.tile as tile
from concourse import bacc, mybir
from concourse import bass_utils

B, T, E = 2, 512, 1024
S, A = 2048, 30
N_CORES = 8
SQ = S // 4  # spans per core
SCH = SQ // 128  # span chunks of 128 partitions
H = E // 2  # half embedding

F32 = mybir.dt.float32
F16 = mybir.dt.float16

NWARM = 15  # PE warm-up matmuls (bridge HAM clock-gate until xq ready)

# packed chunk-0 tile layout: [x0a | Wa | x0b | Wb | b | pad]
XW_W = 2 * E + 8


def _build_body(tc, tch, pattern, out_d, xw_d, x1_d, m_d):
    nc = tc.nc
    AF = mybir.ActivationFunctionType
    OP = mybir.AluOpType

    with (
        tc.tile_pool(name="main", bufs=1) as mainp,
        tc.tile_pool(name="outp", bufs=1) as outp,
        tc.tile_pool(name="psum", bufs=1, space="PSUM") as psp,
    ):
        # Sync ring: packed chunk-0 halves. Scalar ring: x chunk 1
        # (feeds the GpSimd chain), then mask chunks.
        xw = mainp.tile([128, XW_W], F16, name="xw", tag="xw")
        nc.sync.dma_start(xw[:, 0 : 2 * H], xw_d[:, 0 : 2 * H])
        nc.sync.dma_start(xw[:, 2 * H : XW_W], xw_d[:, 2 * H : XW_W])
        x0a = xw[:, 0:H]
        wa = xw[:, H : 2 * H]
        x0b = xw[:, 2 * H : 2 * H + H]
        wb_ = xw[:, 3 * H : 4 * H]
        bb = xw[:, 4 * H : 4 * H + 2].bitcast(F32)

        # All inputs on the sync ring in need-order: FIFO drain gives
        # the first packed half full HBM bandwidth instead of sharing
        # with x chunk 1 on the other ring.
        xt1s = []
        for i in range(1, tch):
            xt = mainp.tile([128, E], F16, name=f"xt{i}", tag=f"xt{i}")
            nc.sync.dma_start(xt[:], x1_d[128 * (i - 1) : 128 * i, :])
            xt1s.append(xt)
        m01s = []
        for i in range(tch):
            mt = mainp.tile([128, SQ], F16, name=f"m{i}", tag=f"m{i}")
            jl = [j for j in range(SCH) if pattern[i][j]]
            lo_c, hi_c = 128 * min(jl), 128 * (max(jl) + 1)
            nc.sync.dma_start(
                mt[:, lo_c:hi_c], m_d[:, SQ * i + lo_c : SQ * i + hi_c]
            )
            m01s.append(mt)

        ones16 = mainp.tile([128, 512], F16)
        nc.vector.memset(ones16[:], 1.0)

        pa = [psp.tile([128, 512], F32, name=f"pa{j}", tag=f"a{j}") for j in range(SCH)]
        zt = [psp.tile([128, 1], F32, name=f"z{j}", tag=f"z{j}") for j in range(SCH)]
        pb = [
            psp.tile([128, 512], F32, name=f"pb{j}", tag=f"z{j}") for j in range(SCH)
        ]

        # PE warm-up sized to end right as the first real matmul's inputs
        # arrive (an overshoot delays real MMs; an undershoot lets the
        # HAM activity window reset and the real MMs run at 1.2 GHz).
        warm = psp.tile([128, 512], F32, name="warm", tag="a0")
        for _ in range(NWARM):
            nc.tensor.matmul(warm[:], ones16[:, 0:128], ones16[:], start=True, stop=True)

        # Head scores, q, xq.  Chunk 0 on DVE, chunk 1 on GpSimd.
        h = mainp.tile([128, 3 * tch], F32, name="h", tag="h")
        expc = mainp.tile([128, tch], F32, name="expc", tag="expc")
        q16 = mainp.tile([128, tch], F16, name="q16", tag="q16")
        qf = mainp.tile([128, tch], F32, name="qf", tag="qf")
        scr = mainp.tile([128, E], F16, name="scr", tag="scr")
        scr1 = mainp.tile([128, E], F16, name="scr1", tag="scr1")
        xqa = [
            mainp.tile([128, H], F16, name=f"xqa{i}", tag=f"xqa{i}") for i in range(tch)
        ]
        xqb = [
            mainp.tile([128, H], F16, name=f"xqb{i}", tag=f"xqb{i}") for i in range(tch)
        ]

        # chunk 0 (DVE)
        nc.vector.scalar_tensor_tensor(
            scr[:, 0:H], x0a, 1.0, wa, op0=OP.mult, op1=OP.mult,
            accum_out=h[:, 0:1],
        )
        nc.vector.scalar_tensor_tensor(
            scr[:, H:E], x0b, 1.0, wb_, op0=OP.mult, op1=OP.mult,
            accum_out=h[:, 1:2],
        )
        with tc.high_priority():
            nc.vector.tensor_tensor(
                h[:, 2 * tch : 2 * tch + 1], h[:, 0:1], h[:, 1:2], op=OP.add
            )
            nc.scalar.activation(
                expc[:, 0:1], h[:, 2 * tch : 2 * tch + 1], AF.Exp, bias=bb
            )
            nc.vector.tensor_scalar_max(q16[:, 0:1], expc[:, 0:1], 1.0)
            nc.vector.tensor_copy(qf[:, 0:1], q16[:, 0:1])
            nc.vector.tensor_scalar_mul(xqa[0][:], x0a, qf[:, 0:1])
            nc.scalar.activation(xqb[0][:], x0b, AF.Copy, scale=qf[:, 0:1])

        # chunks 1.. (DVE halves; the list scheduler interleaves them
        # with chunk 0's q/xq ops as dependencies become ready)
        for i in range(1, tch):
            xt = xt1s[i - 1]
            nc.vector.scalar_tensor_tensor(
                scr1[:, 0:H], xt[:, 0:H], 1.0, wa, op0=OP.mult, op1=OP.mult,
                accum_out=h[:, 2 * i : 2 * i + 1],
            )
            nc.vector.scalar_tensor_tensor(
                scr1[:, H:E], xt[:, H:E], 1.0, wb_, op0=OP.mult, op1=OP.mult,
                accum_out=h[:, 2 * i + 1 : 2 * i + 2],
            )
            nc.vector.tensor_tensor(
                h[:, 2 * tch + i : 2 * tch + i + 1], h[:, 2 * i : 2 * i + 1],
                h[:, 2 * i + 1 : 2 * i + 2], op=OP.add,
            )
            nc.scalar.activation(
                expc[:, i : i + 1], h[:, 2 * tch + i : 2 * tch + i + 1], AF.Exp, bias=bb
            )
            nc.vector.tensor_scalar_max(q16[:, i : i + 1], expc[:, i : i + 1], 1.0)
            nc.vector.tensor_copy(qf[:, i : i + 1], q16[:, i : i + 1])
            nc.vector.tensor_scalar_mul(xqa[i][:], xt[:, 0:H], qf[:, i : i + 1])
            nc.scalar.activation(xqb[i][:], xt[:, H:E], AF.Copy, scale=qf[:, i : i + 1])

        # Matmuls: only (chunk, span-chunk) pairs whose mask slice is
        # nonzero (sorted spans leave early span-chunks entirely inside
        # chunk 0).  Per chunk all live Z's first (the reciprocals that
        # gate every norm need Z closed), then the live A halves; all B
        # halves trail on the recip-freed z banks.
        first_live = [min(i for i in range(tch) if pattern[i][j]) for j in range(SCH)]
        last_live = [max(i for i in range(tch) if pattern[i][j]) for j in range(SCH)]
        rz = mainp.tile([128, SCH], F32, name="rz", tag="rz")
        for i in range(tch):
            for j in range(SCH):
                if not pattern[i][j]:
                    continue
                nc.tensor.matmul(
                    zt[j][:], m01s[i][:, 128 * j : 128 * (j + 1)],
                    q16[:, i : i + 1],
                    start=(i == first_live[j]), stop=(i == last_live[j]),
                )
            for j in range(SCH):
                if not pattern[i][j]:
                    continue
                nc.tensor.matmul(
                    pa[j][:], m01s[i][:, 128 * j : 128 * (j + 1)],
                    xqa[i][:],
                    start=(i == first_live[j]), stop=(i == last_live[j]),
                )
            # j groups fully accumulated at this chunk: reciprocal frees
            # the z bank, then the whole B half runs.
            for j in range(SCH):
                if last_live[j] != i:
                    continue
                nc.vector.reciprocal(rz[:, j : j + 1], zt[j][:, 0:1])
                for i2 in range(tch):
                    if not pattern[i2][j]:
                        continue
                    nc.tensor.matmul(
                        pb[j][:], m01s[i2][:, 128 * j : 128 * (j + 1)],
                        xqb[i2][:],
                        start=(i2 == first_live[j]), stop=(i2 == last_live[j]),
                    )

        # Normalize: one whole-group op per engine (fewer ops amortize
        # the fixed cost; GpSimd cannot read PSUM), store fp16.
        def norm(j, po, lo, eng, ring):
            ob = outp.tile([128, 512], F16, name=f"ob{lo}_{j}", tag=f"ob{lo}_{j}")
            r = rz[:, j : j + 1]
            if eng == "v":
                nc.vector.tensor_scalar_mul(ob[:], po[:], r)
            else:
                nc.scalar.mul(ob[:], po[:], r)
            ring.dma_start(out_d[128 * j : 128 * (j + 1), lo : lo + 512], ob[:])

        # norm in close order, alternating engines
        jorder = sorted(range(SCH), key=lambda j: (last_live[j], j))
        engs = ["s", "s", "s", "s", "v", "v", "v", "v"]
        k = 0
        for j in jorder:
            norm(j, pa[j], 0, engs[k], nc.sync if k % 2 == 0 else nc.scalar)
            k += 1
            norm(j, pb[j], 512, engs[k], nc.sync if k % 2 == 0 else nc.scalar)
            k += 1


def _build(tch, pattern):
    nc = bacc.Bacc(
        "TRN2",
        target_bir_lowering=False,
        debug=False,
        num_devices=N_CORES,
    )
    xw_d = nc.dram_tensor("xw", [128, XW_W], F16, kind="ExternalInput").ap()
    x1_d = nc.dram_tensor("x1", [128 * (tch - 1), E], F16, kind="ExternalInput").ap()
    m_d = nc.dram_tensor("m01", [128, tch * SQ], F16, kind="ExternalInput").ap()
    out_d = nc.dram_tensor("out", [SQ, E], F16, kind="ExternalOutput").ap()
    with tile.TileContext(nc) as tc:
        _build_body(tc, tch, pattern, out_d, xw_d, x1_d, m_d)
    nc.compile()
    return nc


_NC_CACHE = {}


def _get_nc(tch, pattern=None):
    if pattern is None:
        pattern = tuple(tuple(True for _ in range(SCH)) for _ in range(tch))
    key = (tch, pattern)
    if key not in _NC_CACHE:
        _NC_CACHE[key] = _build(tch, pattern)
    return _NC_CACHE[key]


def _make_in_maps(tch, x, W, b, start, end, groups, los):
    """groups[g] = span indices for group g; los[g] = first token of
    g's x window. Each group has exactly SQ spans whose tokens fit in
    [los[g], los[g] + 128*tch)."""
    x = np.asarray(x, dtype=np.float32)
    start = np.asarray(start, dtype=np.int32)
    end = np.asarray(end, dtype=np.int32)
    w16 = np.asarray(W, np.float32).reshape(E).astype(np.float16)
    b16 = np.asarray(b, np.float32).reshape(1).view(np.float16)
    nrow = 128 * tch
    in_maps = []
    for core in range(N_CORES):
        bb_idx, g = divmod(core, 4)
        idx = groups[g]
        lo = los[g]
        hi = min(lo + nrow, T)
        xwin = np.zeros((nrow, E), np.float16)
        xwin[: hi - lo] = x[bb_idx, lo:hi].astype(np.float16)
        xw = np.zeros((128, XW_W), np.float16)
        xw[:, 0:H] = xwin[0:128, 0:H]
        xw[:, H : 2 * H] = w16[None, 0:H]
        xw[:, 2 * H : 3 * H] = xwin[0:128, H:E]
        xw[:, 3 * H : 4 * H] = w16[None, H:E]
        xw[:, 4 * H : 4 * H + 2] = b16[None, :]
        trow = lo + np.arange(nrow, dtype=np.int32)[:, None]
        m = (
            (trow >= start[idx][None, :]) & (trow <= end[idx][None, :])
        ).astype(np.float16)
        m01 = np.empty((128, tch * SQ), np.float16)
        for i in range(tch):
            m01[:, SQ * i : SQ * (i + 1)] = m[128 * i : 128 * (i + 1), :]
        in_maps.append(
            {
                "xw": xw,
                "x1": np.ascontiguousarray(xwin[128:nrow]),
                "m01": np.ascontiguousarray(m01),
            }
        )
    return in_maps


def run(x, W, b, start, end, trace=False, trace_cores=None):
    """Run on 8 cores; returns (out[B,S,E] f32, BassKernelResults)."""
    start_np = np.asarray(start, dtype=np.int32)
    end_np = np.asarray(end, dtype=np.int32)

    # Windowed sharding: sort spans by start, take quarters of 512. Use
    # the K=256 kernel iff every quarter's token span fits 256 rows.
    order = np.argsort(start_np, kind="stable")
    groups = [order[g * SQ : (g + 1) * SQ] for g in range(4)]
    los, ok = [], True
    for idx in groups:
        lo = int(start_np[idx].min())
        hi = int(end_np[idx].max())
        if hi - lo + 1 > 256:
            ok = False
            break
        los.append(min(lo, T - 1))
    if ok:
        tch = 2
    else:
        tch = 4
        groups = [np.arange(g * SQ, (g + 1) * SQ) for g in range(4)]
        los = [0, 0, 0, 0]

    # union live pattern over cores: does chunk i hold any tokens of
    # span-chunk j (128 sorted spans) in any group?
    live = [[False] * SCH for _ in range(tch)]
    for g in range(4):
        idx, lo = groups[g], los[g]
        for j in range(SCH):
            sl = idx[128 * j : 128 * (j + 1)]
            smin = int(start_np[sl].min())
            smax = int(end_np[sl].max())
            for i in range(tch):
                if smax >= lo + 128 * i and smin < lo + 128 * (i + 1):
                    live[i][j] = True
    for j in range(SCH):
        if not any(live[i][j] for i in range(tch)):
            live[0][j] = True
    pattern = tuple(tuple(r) for r in live)
    nc = _get_nc(tch, pattern)
    in_maps = _make_in_maps(tch, x, W, b, start, end, groups, los)
    res = bass_utils.run_bass_kernel_spmd(
        nc,
        in_maps,
        core_ids=list(range(N_CORES)),
        trace=trace,
        trace_cores=trace_cores,
    )
    out = np.empty((B, S, E), np.float32)
    for core in range(N_CORES):
        bb_idx, g = divmod(core, 4)
        out[bb_idx, groups[g]] = res.results[core]["out"].astype(np.float32)
    return out, res


def kernel(x, W, b, start, end):
    out, _ = run(x, W, b, start, end, trace=False)
    return out


# revision 27
# speedup vs baseline: 1.0331x; 1.0331x over previous
"""Trainium2 Bass kernel for span-attention pooling.

Problem shapes (hardcoded):
  x: [B=2, T=512, E=1024] f32, W: [1024, 1] f32, b: [1] f32,
  start/end: [S=2048] i32.  Output: [B, S, E] f32.

Math: out[b,s,:] = sum_t m01[t,s] * q[t] * x[b,t,:] / sum_t m01[t,s] * q[t]
with q = max(exp(h + b), 1) = exp(relu(h + b)), h = x @ W, and
m01[t,s] = (start[s] <= t <= end[s]) a host-precomputed 0/1 mask.
(Equivalent to the reference's per-span softmax over head scores.)

Per core: head scores h via DVE multiply-accumulate halves, q =
max(exp(h+b),1) (ACT+DVE), xq = q * x, then on the PE
  po = m01.T @ xq   (two 512-col halves per 128-span chunk)
  Z  = m01.T @ q16  (N=1 columns)
and out = po * (1/Z) (DVE/ACT whole-group ops), stored fp16 (host
upcasts).  Only (chunk, span-chunk) matmul pairs whose mask slice is
nonzero are emitted (the compiled program is cached per live-pattern);
each span-chunk's poB half and reciprocal fire as soon as its last live
chunk accumulates.

Input packing: chunk 0's x and the replicated W interleave in one tile
([x0a | Wa | x0b | Wb | b]) loaded as two half DMAs on the sync ring, so
the first head-score op starts as soon as 256KB lands; masks follow on
the sync ring while x chunk 1 has the scalar ring to itself.

Sharding: 8 cores = (batch b in {0,1}) x (512-span group). Spans are
sorted by start on the host and split into quarters; since a span covers
at most 30 consecutive tokens, each quarter's spans live inside a window
of <= 256 tokens, so each core only loads its 256-token x slice. If an
exotic span distribution breaks the window property, the kernel falls
back to unsorted quarters with the full K=512.

PSUM bank plan (8): a0..a3 (poA), z0..z3 (Z); each poB reuses its z
bank after the reciprocal reads it. PE warm-up (the HAM clock gate needs
~3.4us of sustained activity and an idle gap resets it) runs dummy
matmuls sized to abut the first real matmul.
"""

import numpy as np

import concourse.bass as bass
import concourse.tile as tile
from concourse import bacc, mybir
from concourse import bass_utils

B, T, E = 2, 512, 1024
S, A = 2048, 30
N_CORES = 8
SQ = S // 4  # spans per core
SCH = SQ // 128  # span chunks of 128 partitions
H = E // 2  # half embedding

F32 = mybir.dt.float32
F16 = mybir.dt.float16

NWARM = 15  # PE warm-up matmuls (bridge HAM clock-gate until xq ready)

# packed chunk-0 tile layout: [x0a | Wa | x0b | Wb | b | pad]
XW_W = 2 * E + 8


def _build_body(tc, tch, pattern, out_d, xw_d, x1_d, m_d):
    nc = tc.nc
    AF = mybir.ActivationFunctionType
    OP = mybir.AluOpType

    with (
        tc.tile_pool(name="main", bufs=1) as mainp,
        tc.tile_pool(name="outp", bufs=1) as outp,
        tc.tile_pool(name="psum", bufs=1, space="PSUM") as psp,
    ):
        # Sync ring: packed chunk-0 halves. Scalar ring: x chunk 1
        # (feeds the GpSimd chain), then mask chunks.
        xw = mainp.tile([128, XW_W], F16, name="xw", tag="xw")
        nc.sync.dma_start(xw[:, 0 : 2 * H], xw_d[:, 0 : 2 * H])
        nc.sync.dma_start(xw[:, 2 * H : XW_W], xw_d[:, 2 * H : XW_W])
        x0a = xw[:, 0:H]
        wa = xw[:, H : 2 * H]
        x0b = xw[:, 2 * H : 2 * H + H]
        wb_ = xw[:, 3 * H : 4 * H]
        bb = xw[:, 4 * H : 4 * H + 2].bitcast(F32)

        xt1s = []
        for i in range(1, tch):
            xt = mainp.tile([128, E], F16, name=f"xt{i}", tag=f"xt{i}")
            nc.scalar.dma_start(xt[:], x1_d[128 * (i - 1) : 128 * i, :])
            xt1s.append(xt)
        m01s = []
        for i in range(tch):
            mt = mainp.tile([128, SQ], F16, name=f"m{i}", tag=f"m{i}")
            jl = [j for j in range(SCH) if pattern[i][j]]
            lo_c, hi_c = 128 * min(jl), 128 * (max(jl) + 1)
            nc.sync.dma_start(
                mt[:, lo_c:hi_c], m_d[:, SQ * i + lo_c : SQ * i + hi_c]
            )
            m01s.append(mt)

        ones16 = mainp.tile([128, 512], F16)
        nc.vector.memset(ones16[:], 1.0)

        pa = [psp.tile([128, 512], F32, name=f"pa{j}", tag=f"a{j}") for j in range(SCH)]
        zt = [psp.tile([128, 1], F32, name=f"z{j}", tag=f"z{j}") for j in range(SCH)]
        pb = [
            psp.tile([128, 512], F32, name=f"pb{j}", tag=f"z{j}") for j in range(SCH)
        ]

        # PE warm-up sized to end right as the first real matmul's inputs
        # arrive (an overshoot delays real MMs; an undershoot lets the
        # HAM activity window reset and the real MMs run at 1.2 GHz).
        warm = psp.tile([128, 512], F32, name="warm", tag="a0")
        for _ in range(NWARM):
            nc.tensor.matmul(warm[:], ones16[:, 0:128], ones16[:], start=True, stop=True)

        # Head scores, q, xq.  Chunk 0 on DVE, chunk 1 on GpSimd.
        h = mainp.tile([128, 3 * tch], F32, name="h", tag="h")
        expc = mainp.tile([128, tch], F32, name="expc", tag="expc")
        q16 = mainp.tile([128, tch], F16, name="q16", tag="q16")
        qf = mainp.tile([128, tch], F32, name="qf", tag="qf")
        scr = mainp.tile([128, E], F16, name="scr", tag="scr")
        scr1 = mainp.tile([128, E], F16, name="scr1", tag="scr1")
        xqa = [
            mainp.tile([128, H], F16, name=f"xqa{i}", tag=f"xqa{i}") for i in range(tch)
        ]
        xqb = [
            mainp.tile([128, H], F16, name=f"xqb{i}", tag=f"xqb{i}") for i in range(tch)
        ]

        # chunk 0 (DVE)
        nc.vector.scalar_tensor_tensor(
            scr[:, 0:H], x0a, 1.0, wa, op0=OP.mult, op1=OP.mult,
            accum_out=h[:, 0:1],
        )
        nc.vector.scalar_tensor_tensor(
            scr[:, H:E], x0b, 1.0, wb_, op0=OP.mult, op1=OP.mult,
            accum_out=h[:, 1:2],
        )
        with tc.high_priority():
            nc.vector.tensor_tensor(
                h[:, 2 * tch : 2 * tch + 1], h[:, 0:1], h[:, 1:2], op=OP.add
            )
            nc.scalar.activation(
                expc[:, 0:1], h[:, 2 * tch : 2 * tch + 1], AF.Exp, bias=bb
            )
            nc.vector.tensor_scalar_max(q16[:, 0:1], expc[:, 0:1], 1.0)
            nc.vector.tensor_copy(qf[:, 0:1], q16[:, 0:1])
            nc.vector.tensor_scalar_mul(xqa[0][:], x0a, qf[:, 0:1])
            nc.scalar.activation(xqb[0][:], x0b, AF.Copy, scale=qf[:, 0:1])

        # chunks 1.. (DVE halves; the list scheduler interleaves them
        # with chunk 0's q/xq ops as dependencies become ready)
        for i in range(1, tch):
            xt = xt1s[i - 1]
            nc.vector.scalar_tensor_tensor(
                scr1[:, 0:H], xt[:, 0:H], 1.0, wa, op0=OP.mult, op1=OP.mult,
                accum_out=h[:, 2 * i : 2 * i + 1],
            )
            nc.vector.scalar_tensor_tensor(
                scr1[:, H:E], xt[:, H:E], 1.0, wb_, op0=OP.mult, op1=OP.mult,
                accum_out=h[:, 2 * i + 1 : 2 * i + 2],
            )
            nc.vector.tensor_tensor(
                h[:, 2 * tch + i : 2 * tch + i + 1], h[:, 2 * i : 2 * i + 1],
                h[:, 2 * i + 1 : 2 * i + 2], op=OP.add,
            )
            nc.scalar.activation(
                expc[:, i : i + 1], h[:, 2 * tch + i : 2 * tch + i + 1], AF.Exp, bias=bb
            )
            nc.vector.tensor_scalar_max(q16[:, i : i + 1], expc[:, i : i + 1], 1.0)
            nc.vector.tensor_copy(qf[:, i : i + 1], q16[:, i : i + 1])
            nc.vector.tensor_scalar_mul(xqa[i][:], xt[:, 0:H], qf[:, i : i + 1])
            nc.scalar.activation(xqb[i][:], xt[:, H:E], AF.Copy, scale=qf[:, i : i + 1])

        # Matmuls: only (chunk, span-chunk) pairs whose mask slice is
        # nonzero (sorted spans leave early span-chunks entirely inside
        # chunk 0).  Per chunk all live Z's first (the reciprocals that
        # gate every norm need Z closed), then the live A halves; all B
        # halves trail on the recip-freed z banks.
        first_live = [min(i for i in range(tch) if pattern[i][j]) for j in range(SCH)]
        last_live = [max(i for i in range(tch) if pattern[i][j]) for j in range(SCH)]
        rz = mainp.tile([128, SCH], F32, name="rz", tag="rz")
        for i in range(tch):
            for j in range(SCH):
                if not pattern[i][j]:
                    continue
                nc.tensor.matmul(
                    zt[j][:], m01s[i][:, 128 * j : 128 * (j + 1)],
                    q16[:, i : i + 1],
                    start=(i == first_live[j]), stop=(i == last_live[j]),
                )
            for j in range(SCH):
                if not pattern[i][j]:
                    continue
                nc.tensor.matmul(
                    pa[j][:], m01s[i][:, 128 * j : 128 * (j + 1)],
                    xqa[i][:],
                    start=(i == first_live[j]), stop=(i == last_live[j]),
                )
            # j groups fully accumulated at this chunk: reciprocal frees
            # the z bank, then the whole B half runs.
            for j in range(SCH):
                if last_live[j] != i:
                    continue
                nc.vector.reciprocal(rz[:, j : j + 1], zt[j][:, 0:1])
                for i2 in range(tch):
                    if not pattern[i2][j]:
                        continue
                    nc.tensor.matmul(
                        pb[j][:], m01s[i2][:, 128 * j : 128 * (j + 1)],
                        xqb[i2][:],
                        start=(i2 == first_live[j]), stop=(i2 == last_live[j]),
                    )

        # Normalize: one whole-group op per engine (fewer ops amortize
        # the fixed cost; GpSimd cannot read PSUM), store fp16.
        def norm(j, po, lo, eng, ring):
            ob = outp.tile([128, 512], F16, name=f"ob{lo}_{j}", tag=f"ob{lo}_{j}")
            r = rz[:, j : j + 1]
            if eng == "v":
                nc.vector.tensor_scalar_mul(ob[:], po[:], r)
            else:
                nc.scalar.mul(ob[:], po[:], r)
            ring.dma_start(out_d[128 * j : 128 * (j + 1), lo : lo + 512], ob[:])

        # norm in close order, alternating engines
        jorder = sorted(range(SCH), key=lambda j: (last_live[j], j))
        engs = ["s", "s", "s", "s", "v", "v", "v", "v"]
        k = 0
        for j in jorder:
            norm(j, pa[j], 0, engs[k], nc.sync if k % 2 == 0 else nc.scalar)
            k += 1
            norm(j, pb[j], 512, engs[k], nc.sync if k % 2 == 0 else nc.scalar)
            k += 1


def _build(tch, pattern):
    nc = bacc.Bacc(
        "TRN2",
        target_bir_lowering=False,
        debug=False,
        num_devices=N_CORES,
    )
    xw_d = nc.dram_tensor("xw", [128, XW_W], F16, kind="ExternalInput").ap()
    x1_d = nc.dram_tensor("x1", [128 * (tch - 1), E], F16, kind="ExternalInput").ap()
    m_d = nc.dram_tensor("m01", [128, tch * SQ], F16, kind="ExternalInput").ap()
    out_d = nc.dram_tensor("out", [SQ, E], F16, kind="ExternalOutput").ap()
    with tile.TileContext(nc) as tc:
        _build_body(tc, tch, pattern, out_d, xw_d, x1_d, m_d)
    nc.compile()
    return nc


_NC_CACHE = {}


def _get_nc(tch, pattern=None):
    if pattern is None:
        pattern = tuple(tuple(True for _ in range(SCH)) for _ in range(tch))
    key = (tch, pattern)
    if key not in _NC_CACHE:
        _NC_CACHE[key] = _build(tch, pattern)
    return _NC_CACHE[key]


def _make_in_maps(tch, x, W, b, start, end, groups, los):
    """groups[g] = span indices for group g; los[g] = first token of
    g's x window. Each group has exactly SQ spans whose tokens fit in
    [los[g], los[g] + 128*tch)."""
    x = np.asarray(x, dtype=np.float32)
    start = np.asarray(start, dtype=np.int32)
    end = np.asarray(end, dtype=np.int32)
    w16 = np.asarray(W, np.float32).reshape(E).astype(np.float16)
    b16 = np.asarray(b, np.float32).reshape(1).view(np.float16)
    nrow = 128 * tch
    in_maps = []
    for core in range(N_CORES):
        bb_idx, g = divmod(core, 4)
        idx = groups[g]
        lo = los[g]
        hi = min(lo + nrow, T)
        xwin = np.zeros((nrow, E), np.float16)
        xwin[: hi - lo] = x[bb_idx, lo:hi].astype(np.float16)
        xw = np.zeros((128, XW_W), np.float16)
        xw[:, 0:H] = xwin[0:128, 0:H]
        xw[:, H : 2 * H] = w16[None, 0:H]
        xw[:, 2 * H : 3 * H] = xwin[0:128, H:E]
        xw[:, 3 * H : 4 * H] = w16[None, H:E]
        xw[:, 4 * H : 4 * H + 2] = b16[None, :]
        trow = lo + np.arange(nrow, dtype=np.int32)[:, None]
        m = (
            (trow >= start[idx][None, :]) & (trow <= end[idx][None, :])
        ).astype(np.float16)
        m01 = np.empty((128, tch * SQ), np.float16)
        for i in range(tch):
            m01[:, SQ * i : SQ * (i + 1)] = m[128 * i : 128 * (i + 1), :]
        in_maps.append(
            {
                "xw": xw,
                "x1": np.ascontiguousarray(xwin[128:nrow]),
                "m01": np.ascontiguousarray(m01),
            }
        )
    return in_maps


def run(x, W, b, start, end, trace=False, trace_cores=None):
    """Run on 8 cores; returns (out[B,S,E] f32, BassKernelResults)."""
    start_np = np.asarray(start, dtype=np.int32)
    end_np = np.asarray(end, dtype=np.int32)

    # Windowed sharding: sort spans by start, take quarters of 512. Use
    # the K=256 kernel iff every quarter's token span fits 256 rows.
    order = np.argsort(start_np, kind="stable")
    groups = [order[g * SQ : (g + 1) * SQ] for g in range(4)]
    los, ok = [], True
    for idx in groups:
        lo = int(start_np[idx].min())
        hi = int(end_np[idx].max())
        if hi - lo + 1 > 256:
            ok = False
            break
        los.append(min(lo, T - 1))
    if ok:
        tch = 2
    else:
        tch = 4
        groups = [np.arange(g * SQ, (g + 1) * SQ) for g in range(4)]
        los = [0, 0, 0, 0]

    # union live pattern over cores: does chunk i hold any tokens of
    # span-chunk j (128 sorted spans) in any group?
    live = [[False] * SCH for _ in range(tch)]
    for g in range(4):
        idx, lo = groups[g], los[g]
        for j in range(SCH):
            sl = idx[128 * j : 128 * (j + 1)]
            smin = int(start_np[sl].min())
            smax = int(end_np[sl].max())
            for i in range(tch):
                if smax >= lo + 128 * i and smin < lo + 128 * (i + 1):
                    live[i][j] = True
    for j in range(SCH):
        if not any(live[i][j] for i in range(tch)):
            live[0][j] = True
    pattern = tuple(tuple(r) for r in live)
    nc = _get_nc(tch, pattern)
    in_maps = _make_in_maps(tch, x, W, b, start, end, groups, los)
    res = bass_utils.run_bass_kernel_spmd(
        nc,
        in_maps,
        core_ids=list(range(N_CORES)),
        trace=trace,
        trace_cores=trace_cores,
    )
    out = np.empty((B, S, E), np.float32)
    for core in range(N_CORES):
        bb_idx, g = divmod(core, 4)
        out[bb_idx, groups[g]] = res.results[core]["out"].astype(np.float32)
    return out, res


def kernel(x, W, b, start, end):
    out, _ = run(x, W, b, start, end, trace=False)
    return out
